# revision 10
# baseline (speedup 1.0000x reference)
"""HAN layer (4 metapaths x 2-layer mean-RGCN + metapath attention) on 8 trn2 cores.

Sharding: cores (2i, 2i+1) handle metapath i. Within a pair, L1 splits dst into
halves [0,nreg)/[nreg,2*nreg); after an in-pair AllGather of x1, L2 splits the
NREG range into quarters. Attention: score AllGather + ReduceScatter over the 4
cores holding the same node range ({0,2,4,6} and {1,3,5,7}).

Device algorithm per layer (linearity: segment_sum(x[src]) @ Wm): edges are
host-sorted by dst into groups of 128 dsts; an indirect DMA gathers x[src] rows
for a group; per 128-edge chunk a selector eq[e,d] = (dl[e]==d)*rec[e] is built
on DVE and matmul-accumulated on PE into meanT = (segment_mean)^T in PSUM; two
dense matmuls + fused ReLU produce the group's 128 output rows, written
contiguously (no scatter anywhere).
"""

import math
import numpy as np

import concourse.bass as bass
import concourse.bacc as bacc
import concourse.mybir as mybir
from concourse.tile import TileContext
from concourse.bass_utils import run_bass_kernel_spmd

F32 = mybir.dt.float32
I32 = mybir.dt.int32

N_CORES = 8
BF = 4     # output groups batched per store DMA
CH = 16    # groups per grid-load DMA


# ----------------------------------------------------------------- host prep

def _build_grids(srcs, dsts, lo, ng, nb, rec):
    """grid[p, g*nb + b] = edge at (partition p, chunk b) of group g; the
    indirect-DMA flat order j = p*nb + b lands row j at out-partition p,
    column block b. Empty slots: dl=128 (selector row all-zero)."""
    g = (dsts - lo) >> 7
    starts = np.searchsorted(dsts, lo + 128 * np.arange(ng))
    slot = np.arange(len(dsts)) - starts[g]
    p = slot & 127
    b = slot >> 7
    col = g * nb + b
    idx_g = np.zeros((128, nb * ng), np.int32)
    dl_g = np.full((128, nb * ng), 128.0, np.float32)
    rec_g = np.zeros((128, nb * ng), np.float32)
    idx_g[p, col] = srcs
    dl_g[p, col] = (dsts - lo - (g << 7)).astype(np.float32)
    rec_g[p, col] = rec[dsts]
    return idx_g, dl_g, rec_g


def _group_max(dsts, lo, ng):
    starts = np.searchsorted(dsts, lo + 128 * np.arange(ng + 1))
    return int(np.diff(starts).max()) if len(dsts) else 1


# ------------------------------------------------------------- device build

def _emit_layer(nc, tc, pools, table, gidx, gdl, grec, gidxd, wm_t, wr_t,
                ng, nb, iota_t, ident_t, out_dram, rows_total, hook=None):
    sb, sbg, psum, sbeq = pools
    nch = math.ceil(ng / CH)
    stage = None
    for g in range(ng):
        if g % CH == 0:
            w = min(CH, ng - g)
            idxt = sbg.tile([128, nb * w], I32, tag="idxt")
            nc.sync.dma_start(out=idxt[:], in_=gidx[:, g * nb:(g + w) * nb])
            dlt = sbg.tile([128, nb * w], F32, tag="dlt")
            nc.sync.dma_start(out=dlt[:], in_=gdl[:, g * nb:(g + w) * nb])
            rect = sbg.tile([128, nb * w], F32, tag="rect")
            nc.sync.dma_start(out=rect[:], in_=grec[:, g * nb:(g + w) * nb])
            idxdt = sbg.tile([128, w], I32, tag="idxdt")
            nc.sync.dma_start(out=idxdt[:], in_=gidxd[:, g:g + w])
        o = (g % CH) * nb

        msgs = sb.tile([128, nb * 128], F32, tag="msgs")
        for b in range(nb):
            nc.gpsimd.indirect_dma_start(
                out=msgs[:, b * 128:(b + 1) * 128], out_offset=None,
                in_=table[:],
                in_offset=bass.IndirectOffsetOnAxis(
                    ap=idxt[:, o + b:o + b + 1], axis=0))

        meant_ps = psum.tile([128, 128], F32, space="PSUM", tag="meant")
        for b in range(nb):
            eq = sbeq.tile([128, 128], F32, tag="eq")
            nc.vector.tensor_scalar(
                out=eq[:], in0=iota_t[:],
                scalar1=dlt[:, o + b:o + b + 1], scalar2=rect[:, o + b:o + b + 1],
                op0=mybir.AluOpType.is_equal, op1=mybir.AluOpType.mult)
            nc.tensor.matmul(out=meant_ps[:], lhsT=msgs[:, b * 128:(b + 1) * 128],
                             rhs=eq[:], start=(b == 0), stop=(b == nb - 1))
        meant = sb.tile([128, 128], F32, tag="meant_sb")
        nc.vector.tensor_copy(out=meant[:], in_=meant_ps[:])

        xd = sb.tile([128, 128], F32, tag="xd")
        nc.gpsimd.indirect_dma_start(
            out=xd[:], out_offset=None, in_=table[:],
            in_offset=bass.IndirectOffsetOnAxis(
                ap=idxdt[:, g % CH:g % CH + 1], axis=0))
        xdt_ps = psum.tile([128, 128], F32, space="PSUM", tag="xdt")
        nc.tensor.transpose(out=xdt_ps[:], in_=xd[:], identity=ident_t[:])
        xdt = sb.tile([128, 128], F32, tag="xdt_sb")
        nc.vector.tensor_copy(out=xdt[:], in_=xdt_ps[:])

        h_ps = psum.tile([128, 128], F32, space="PSUM", tag="hps")
        nc.tensor.matmul(out=h_ps[:], lhsT=meant[:], rhs=wm_t[:],
                         start=True, stop=False)
        nc.tensor.matmul(out=h_ps[:], lhsT=xdt[:], rhs=wr_t[:],
                         start=False, stop=True)

        gb = g % BF
        if gb == 0:
            bw = min(BF, ng - g)
            stage = sb.tile([128, bw * 128], F32, tag="xn_stage")
        xn = stage[:, gb * 128:(gb + 1) * 128]
        nc.scalar.activation(out=xn, in_=h_ps[:],
                             func=mybir.ActivationFunctionType.Relu)
        if hook is not None:
            hook(g, xn)
        if gb == bw - 1:
            g0 = g - gb
            rows = min((gb + 1) * 128, rows_total - g0 * 128)
            nfull = rows // 128
            if nfull > 0:
                nc.sync.dma_start(
                    out=out_dram[g0 * 128:g0 * 128 + nfull * 128, :]
                    .rearrange("(a t) f -> t a f", t=128),
                    in_=stage[:, :nfull * 128]
                    .rearrange("p (a f) -> p a f", f=128))
            rem = rows - nfull * 128
            if rem > 0:
                nc.sync.dma_start(
                    out=out_dram[g0 * 128 + nfull * 128:
                                 g0 * 128 + nfull * 128 + rem, :],
                    in_=stage[:rem, nfull * 128:(nfull + 1) * 128])


def build_program(n, nreg, ng1, nb1, ng2, nb2, debug=False):
    nc = bacc.Bacc("TRN2", target_bir_lowering=False, debug=False,
                   num_devices=N_CORES)
    half = nreg
    nrs = (ng2 * 128) // 4  # ReduceScatter rows per rank

    ei = lambda name, shape, dt=F32: nc.dram_tensor(name, shape, dt,
                                                    kind="ExternalInput")
    x0 = ei("x0", [n, 128])
    g1_idx = ei("g1_idx", [128, nb1 * ng1], I32)
    g1_dl = ei("g1_dl", [128, nb1 * ng1])
    g1_rec = ei("g1_rec", [128, nb1 * ng1])
    g1_idxd = ei("g1_idxd", [128, ng1], I32)
    g2_idx = ei("g2_idx", [128, nb2 * ng2], I32)
    g2_dl = ei("g2_dl", [128, nb2 * ng2])
    g2_rec = ei("g2_rec", [128, nb2 * ng2])
    g2_idxd = ei("g2_idxd", [128, ng2], I32)
    wm1, wr1 = ei("wm1", [128, 128]), ei("wr1", [128, 128])
    wm2, wr2 = ei("wm2", [128, 128]), ei("wr2", [128, 128])
    qs_rep = ei("qs_rep", [128, 128])
    sel = ei("sel", [128, 4])
    iota_in = ei("iota", [128, 128])
    ident_in = ei("ident", [128, 128])

    out_part = nc.dram_tensor("out_part", [nrs, 128], F32,
                              kind="ExternalOutput")

    x1_half = nc.dram_tensor("x1_half", [half, 128], F32)
    x1_full = nc.dram_tensor("x1_full", [n, 128], F32)
    x2b = nc.dram_tensor("x2b", [ng2 * 128, 128], F32)
    sc_in = nc.dram_tensor("sc_in", [ng2, 128], F32)
    sc_all = nc.dram_tensor("sc_all", [4 * ng2, 128], F32)
    rs_in = nc.dram_tensor("rs_in", [ng2 * 128, 128], F32)
    rs_out = nc.dram_tensor("rs_out", [nrs, 128], F32)

    pair_groups = [[2 * i, 2 * i + 1] for i in range(4)]
    attn_groups = [[0, 2, 4, 6], [1, 3, 5, 7]]

    with TileContext(nc) as tc:
        with (
            tc.tile_pool(name="const", bufs=1) as cpool,
            tc.tile_pool(name="sb", bufs=3) as sb,
            tc.tile_pool(name="sbg", bufs=2) as sbg,
            tc.tile_pool(name="sbeq", bufs=4) as sbeq,
            tc.tile_pool(name="psum", bufs=2, space="PSUM") as psum,
        ):
            def cload(src, shape, tag):
                t = cpool.tile(shape, F32, tag=tag)
                nc.sync.dma_start(out=t[:], in_=src[:, :])
                return t

            iota_t = cload(iota_in, [128, 128], "c_iota")
            ident_t = cload(ident_in, [128, 128], "c_ident")
            wm1_t = cload(wm1, [128, 128], "c_wm1")
            wr1_t = cload(wr1, [128, 128], "c_wr1")
            wm2_t = cload(wm2, [128, 128], "c_wm2")
            wr2_t = cload(wr2, [128, 128], "c_wr2")
            qs_t = cload(qs_rep, [128, 128], "c_qs")
            sel_t = cload(sel, [128, 4], "c_sel")
            score_sb = cpool.tile([128, ng2], F32, tag="c_score")

            pools = (sb, sbg, psum, sbeq)

            _emit_layer(nc, tc, pools, x0, g1_idx, g1_dl, g1_rec, g1_idxd,
                        wm1_t, wr1_t, ng1, nb1, iota_t, ident_t,
                        x1_half, half)

            nc.gpsimd.collective_compute(
                "AllGather", mybir.AluOpType.bypass,
                replica_groups=pair_groups,
                ins=[x1_half[:, :]], outs=[x1_full[:, :]])

            def score_hook(g, xn):
                t = sb.tile([128, 128], F32, tag="sc_tmp")
                nc.vector.tensor_tensor(out=t[:], in0=xn, in1=qs_t[:],
                                        op=mybir.AluOpType.mult)
                nc.vector.reduce_sum(out=score_sb[:, g:g + 1], in_=t[:],
                                     axis=mybir.AxisListType.X)

            _emit_layer(nc, tc, pools, x1_full, g2_idx, g2_dl, g2_rec, g2_idxd,
                        wm2_t, wr2_t, ng2, nb2, iota_t, ident_t,
                        x2b, ng2 * 128, hook=score_hook)

            nc.sync.dma_start(out=sc_in[:, :].rearrange("t p -> p t"),
                              in_=score_sb[:, :])
            nc.gpsimd.collective_compute(
                "AllGather", mybir.AluOpType.bypass,
                replica_groups=attn_groups,
                ins=[sc_in[:, :]], outs=[sc_all[:, :]])

            # softmax over 4 metapaths (elementwise across four [128,ng2] tiles)
            s_t = []
            for p in range(4):
                st = cpool.tile([128, ng2], F32, tag=f"s{p}")
                nc.sync.dma_start(
                    out=st[:],
                    in_=sc_all[p * ng2:(p + 1) * ng2, :].rearrange("t p -> p t"))
                s_t.append(st)
            m = cpool.tile([128, ng2], F32, tag="c_m")
            nc.vector.tensor_tensor(out=m[:], in0=s_t[0][:], in1=s_t[1][:],
                                    op=mybir.AluOpType.max)
            for p in (2, 3):
                nc.vector.tensor_tensor(out=m[:], in0=m[:], in1=s_t[p][:],
                                        op=mybir.AluOpType.max)
            e_t = []
            for p in range(4):
                dt_ = cpool.tile([128, ng2], F32, tag=f"d{p}")
                nc.vector.tensor_tensor(out=dt_[:], in0=s_t[p][:], in1=m[:],
                                        op=mybir.AluOpType.subtract)
                et = cpool.tile([128, ng2], F32, tag=f"e{p}")
                nc.scalar.activation(out=et[:], in_=dt_[:],
                                     func=mybir.ActivationFunctionType.Exp)
                e_t.append(et)
            z = cpool.tile([128, ng2], F32, tag="c_z")
            nc.vector.tensor_tensor(out=z[:], in0=e_t[0][:], in1=e_t[1][:],
                                    op=mybir.AluOpType.add)
            for p in (2, 3):
                nc.vector.tensor_tensor(out=z[:], in0=z[:], in1=e_t[p][:],
                                        op=mybir.AluOpType.add)
            rz = cpool.tile([128, ng2], F32, tag="c_rz")
            nc.vector.reciprocal(out=rz[:], in_=z[:])
            wown = cpool.tile([128, ng2], F32, tag="c_wown")
            acc = cpool.tile([128, ng2], F32, tag="c_acc")
            nc.vector.tensor_scalar(out=wown[:], in0=e_t[0][:],
                                    scalar1=sel_t[:, 0:1], scalar2=None,
                                    op0=mybir.AluOpType.mult)
            for p in (1, 2, 3):
                nc.vector.tensor_scalar(out=acc[:], in0=e_t[p][:],
                                        scalar1=sel_t[:, p:p + 1], scalar2=None,
                                        op0=mybir.AluOpType.mult)
                nc.vector.tensor_tensor(out=wown[:], in0=wown[:], in1=acc[:],
                                        op=mybir.AluOpType.add)
            nc.vector.tensor_tensor(out=wown[:], in0=wown[:], in1=rz[:],
                                    op=mybir.AluOpType.mult)

            # weighted partials, batched BF groups per DMA
            for g0 in range(0, ng2, BF):
                bw = min(BF, ng2 - g0)
                xt = sb.tile([128, bw * 128], F32, tag="attn_x")
                nc.sync.dma_start(
                    out=xt[:].rearrange("p (a f) -> p a f", f=128),
                    in_=x2b[g0 * 128:(g0 + bw) * 128, :]
                    .rearrange("(a t) f -> t a f", t=128))
                wt = sb.tile([128, bw * 128], F32, tag="attn_w")
                for j in range(bw):
                    nc.vector.tensor_scalar(
                        out=wt[:, j * 128:(j + 1) * 128],
                        in0=xt[:, j * 128:(j + 1) * 128],
                        scalar1=wown[:, g0 + j:g0 + j + 1], scalar2=None,
                        op0=mybir.AluOpType.mult)
                nc.sync.dma_start(
                    out=rs_in[g0 * 128:(g0 + bw) * 128, :]
                    .rearrange("(a t) f -> t a f", t=128),
                    in_=wt[:].rearrange("p (a f) -> p a f", f=128))

            nc.gpsimd.collective_compute(
                "ReduceScatter", mybir.AluOpType.add,
                replica_groups=attn_groups,
                ins=[rs_in[:, :]], outs=[rs_out[:, :]])

            # rs_out [nrs,128] -> out_part, bounced through SBUF
            nblk = nrs // 128
            fin = cpool.tile([128, nblk * 128], F32, tag="c_fin")
            nc.sync.dma_start(
                out=fin[:].rearrange("p (a f) -> p a f", f=128),
                in_=rs_out[:, :].rearrange("(a t) f -> t a f", t=128))
            nc.sync.dma_start(
                out=out_part[:, :].rearrange("(a t) f -> t a f", t=128),
                in_=fin[:].rearrange("p (a f) -> p a f", f=128))

            if debug:
                def dump(src, dst, rows):
                    for r0 in range(0, rows, 128):
                        r = min(128, rows - r0)
                        t = sb.tile([128, 128], F32, tag="dbg")
                        nc.sync.dma_start(out=t[:r, :], in_=src[r0:r0 + r, :])
                        nc.sync.dma_start(out=dst[r0:r0 + r, :], in_=t[:r, :])
                dbg_x1 = nc.dram_tensor("dbg_x1", [n, 128], F32,
                                        kind="ExternalOutput")
                dump(x1_full, dbg_x1, n)
                dbg_x2 = nc.dram_tensor("dbg_x2", [ng2 * 128, 128], F32,
                                        kind="ExternalOutput")
                dump(x2b, dbg_x2, ng2 * 128)
                dbg_sc = nc.dram_tensor("dbg_sc", [4 * ng2, 128], F32,
                                        kind="ExternalOutput")
                dump(sc_all, dbg_sc, 4 * ng2)
                dbg_w = nc.dram_tensor("dbg_w", [128, ng2], F32,
                                       kind="ExternalOutput")
                wt_ = sb.tile([128, ng2], F32, tag="dbg_w")
                nc.vector.tensor_copy(out=wt_[:], in_=wown[:])
                nc.sync.dma_start(out=dbg_w[:, :], in_=wt_[:])
    return nc


# ----------------------------------------------------------------- kernel()

def kernel(E, metapath_emb, W_root, W_rel, b, Wq, bq, edge_index, eids,
           nreg=50000, trace=False, debug=False):
    P = edge_index.shape[0]
    n = eids.shape[1]
    d = E.shape[1]
    scale = np.float32(1.0 / math.sqrt(d))
    assert P == 4 and d == 128 and n == 2 * nreg and nreg % 4 == 0
    assert not np.any(np.asarray(b)), "nonzero bias not supported"

    E = np.asarray(E, np.float32)
    edge_index = np.asarray(edge_index)
    eids = np.asarray(eids)

    query = (np.asarray(metapath_emb, np.float32) @ np.asarray(Wq, np.float32)
             + np.asarray(bq, np.float32))
    query_scaled = query * scale

    ng1 = math.ceil(nreg / 128)
    ng2 = math.ceil((nreg // 2) / 128)

    # per-metapath: x0, degree recip, dst-sorted edges
    metas = []
    for i in range(P):
        src = edge_index[i, 0].astype(np.int32)
        dst = edge_index[i, 1].astype(np.int32)
        x0 = np.ascontiguousarray(E[eids[i]]).astype(np.float32)
        deg = np.bincount(dst, minlength=n).astype(np.float32)
        rec = (1.0 / np.maximum(deg, 1.0)).astype(np.float32)
        order = np.argsort(dst, kind="stable")
        metas.append((x0, rec, src[order], dst[order]))

    def rng(i, lo, hi):
        _, _, ssrc, sdst = metas[i]
        a, bb = np.searchsorted(sdst, [lo, hi])
        return ssrc[a:bb], sdst[a:bb]

    spans = []
    for c in range(N_CORES):
        i, h = c // 2, c % 2
        lo1, lo2 = h * nreg, h * (nreg // 2)
        spans.append((rng(i, lo1, lo1 + ng1 * 128),
                      rng(i, lo2, lo2 + ng2 * 128), lo1, lo2))

    nb1 = max(1, max(math.ceil(_group_max(s[0][1], s[2], ng1) / 128)
                     for s in spans))
    nb2 = max(1, max(math.ceil(_group_max(s[1][1], s[3], ng2) / 128)
                     for s in spans))

    iota = np.tile(np.arange(128, dtype=np.float32), (128, 1))
    ident = np.eye(128, dtype=np.float32)

    in_maps = []
    for c in range(N_CORES):
        i, h = c // 2, c % 2
        (s1, d1), (s2, d2), lo1, lo2 = spans[c]
        rec = metas[i][1]
        i1, l1, r1 = _build_grids(s1, d1, lo1, ng1, nb1, rec)
        i2, l2, r2 = _build_grids(s2, d2, lo2, ng2, nb2, rec)
        idxd1 = np.minimum(lo1 + 128 * np.arange(ng1)[None, :]
                           + np.arange(128)[:, None], n - 1).astype(np.int32)
        idxd2 = np.minimum(lo2 + 128 * np.arange(ng2)[None, :]
                           + np.arange(128)[:, None], n - 1).astype(np.int32)
        selm = np.zeros((128, 4), np.float32)
        selm[:, i] = 1.0
        in_maps.append(dict(
            x0=metas[i][0], g1_idx=i1, g1_dl=l1, g1_rec=r1,
            g1_idxd=np.ascontiguousarray(idxd1),
            g2_idx=i2, g2_dl=l2, g2_rec=r2,
            g2_idxd=np.ascontiguousarray(idxd2),
            wm1=np.ascontiguousarray(W_rel[i, 0]).astype(np.float32),
            wr1=np.ascontiguousarray(W_root[i, 0]).astype(np.float32),
            wm2=np.ascontiguousarray(W_rel[i, 1]).astype(np.float32),
            wr2=np.ascontiguousarray(W_root[i, 1]).astype(np.float32),
            qs_rep=np.tile(query_scaled[i], (128, 1)).astype(np.float32),
            sel=selm, iota=iota, ident=ident,
        ))

    nc = build_program(n, nreg, ng1, nb1, ng2, nb2, debug=debug)
    nc.compile()
    kernel.last_nc = nc
    kernel.last_in_maps = in_maps
    res = run_bass_kernel_spmd(nc, in_maps, core_ids=list(range(N_CORES)),
                               trace=trace)

    q = nreg // 2
    a_rows = np.concatenate([res.results[c]["out_part"] for c in (0, 2, 4, 6)],
                            axis=0)[:q]
    b_rows = np.concatenate([res.results[c]["out_part"] for c in (1, 3, 5, 7)],
                            axis=0)[:q]
    out = np.concatenate([a_rows, b_rows], axis=0).astype(np.float32)
    kernel.last_results = res
    return out
